# revision 1
# baseline (speedup 1.0000x reference)
"""Trainium2 Bass kernel for nn_MultiHeadAttention (B=2, S=2048, D=1024, H=16).

Sharding: 8 cores = 2 batches x 4 head-groups. Core c handles batch c//4 and
heads [4*(c%4), 4*(c%4)+4). Each core computes its 4 heads' attention plus the
row-slice of the output projection; the host sums the 4 partial outputs per
batch and adds the output bias.

Per-core layout (transpose-free attention):
  - qT/kT computed in [head_dim, seq] layout (contraction over D needs x^T,
    which the host provides), packed 2 heads per 128-partition tile.
  - scoresT[kv, q] = kT.T @ qT per (head, kv-tile, q-chunk); exp on ScalarE
    reading PSUM directly (scale=1/8 folded into the activation).
  - attnT'[d+1, q] = [v | 1]^T.T @ exp_scoresT accumulated over kv in PSUM:
    row 64 collects the softmax denominators for free (ones column in v').
  - recip = exp(-ln(sums)) on ScalarE (same activation table set as exp);
    broadcast across partitions with a f32r ones-column matmul; DVE multiply
    normalizes attnT into bf16 SBUF.
  - out[s, :] = attnT.T @ wo accumulated over the 4 heads (K=64 each).

All matmuls run in bf16 (inputs cast on host) with fp32 PSUM accumulation.
"""

import sys

for _p in ("/opt/trn_rl_repo",):
    if _p not in sys.path:
        sys.path.insert(0, _p)

import numpy as np
import ml_dtypes

BF16 = ml_dtypes.bfloat16

S = 2048          # sequence length
D = 1024          # embed dim
HC = 4            # heads per core
HD = 64           # head dim
DC = HC * HD      # per-core projection width (256)
ST = S // 128     # s-tiles (16)
DT = D // 128     # D-tiles (8)
QC = S // 512     # q-chunks of 512 (4)
NCORES = 8

_PROGRAM = None


def _build_program():
    import concourse.mybir as mybir
    import concourse.tile as tile
    from concourse import bacc

    dt = mybir.dt
    AF = mybir.ActivationFunctionType
    ALU = mybir.AluOpType

    class _Bacc(bacc.Bacc):
        def insert_act_table_loads(self):
            # This kernel only uses Exp and Ln on the scalar engine; steer the
            # table-load pass to the combined natural_log_exp_and_others set
            # (one resident table, zero mid-stream reloads) by blanking the
            # exp-only / ln-only sets. Indices must be preserved, so entries
            # are emptied rather than removed.
            from concourse.hw_specs import get_activation_tables

            has_activation = any(
                isinstance(i, mybir.InstActivation)
                for b in self.main_func.blocks
                for i in b.instructions
            )
            if not has_activation:
                return
            tables = []
            for name, funcs in get_activation_tables(self.m.arch).items():
                if name in ("exp_and_others", "exp_and_friends", "natural_log"):
                    funcs = set()
                tables.append((name, funcs))
            bacc._bass_rust.insert_act_table_loads(self, tables)

    nc = _Bacc()

    xqT = nc.declare_dram_parameter("xqT", [D, S], dt.bfloat16, isOutput=False)
    xkT = nc.declare_dram_parameter("xkT", [D, S], dt.bfloat16, isOutput=False)
    xvT = nc.declare_dram_parameter("xvT", [D, S], dt.bfloat16, isOutput=False)
    wq = nc.declare_dram_parameter("wq", [D, DC], dt.bfloat16, isOutput=False)
    wk = nc.declare_dram_parameter("wk", [D, DC], dt.bfloat16, isOutput=False)
    wv = nc.declare_dram_parameter("wv", [D, DC], dt.bfloat16, isOutput=False)
    wo = nc.declare_dram_parameter("wo", [HD, HC, D], dt.bfloat16, isOutput=False)
    bq = nc.declare_dram_parameter("bq", [128, 2], dt.float32, isOutput=False)
    bk = nc.declare_dram_parameter("bk", [128, 2], dt.float32, isOutput=False)
    bv = nc.declare_dram_parameter("bv", [128, DC], dt.float32, isOutput=False)
    ones = nc.declare_dram_parameter("ones", [128, 128], dt.float32r, isOutput=False)
    out = nc.declare_dram_parameter("out", [S, D], dt.float32, isOutput=True)

    out_t = out.rearrange("(t p) d -> t p d", p=128)

    with tile.TileContext(nc) as tc:
        with (
            tc.tile_pool(name="const", bufs=1) as cp,
            tc.tile_pool(name="xt", bufs=34) as xp,
            tc.tile_pool(name="expp", bufs=24) as ep,
            tc.tile_pool(name="atp", bufs=3) as atp,
            tc.tile_pool(name="rcp", bufs=2) as rcp,
            tc.tile_pool(name="outp", bufs=4) as op_,
            tc.tile_pool(name="pa", bufs=2, space="PSUM") as pa,
            tc.tile_pool(name="pb", bufs=4, space="PSUM") as pb,
        ):
            # ---- constants ----
            wq_sb = cp.tile([128, DT, DC], dt.bfloat16, tag="wq_sb")
            wk_sb = cp.tile([128, DT, DC], dt.bfloat16, tag="wk_sb")
            wv_sb = cp.tile([128, DT, DC], dt.bfloat16, tag="wv_sb")
            wo_sb = cp.tile([HD, HC, D], dt.bfloat16, tag="wo_sb")
            bq_sb = cp.tile([128, 2], dt.float32, tag="bq_sb")
            bk_sb = cp.tile([128, 2], dt.float32, tag="bk_sb")
            bv_sb = cp.tile([128, DC], dt.float32, tag="bv_sb")
            ones_sb = cp.tile([128, 128], dt.float32r, tag="ones_sb")
            # only K/Q weights go ahead of the critical xk/xq input stream;
            # wv/wo/ones are deferred until after the first input halves
            nc.sync.dma_start(wk_sb[:], wk.rearrange("(t p) m -> p t m", p=128))
            nc.sync.dma_start(bk_sb[:], bk[:])

            # q/k/v staged as per-chunk tiles so attention's dependencies are
            # fine-grained (a monolithic tile would stall attention until the
            # last projection write).
            qT_sb = [cp.tile([128, 2, 512], dt.bfloat16, tag=f"qT_sb{i}", name=f"qT_sb{i}") for i in range(QC)]
            kT_sb = [cp.tile([128, 2, 512], dt.bfloat16, tag=f"kT_sb{i}", name=f"kT_sb{i}") for i in range(QC)]
            # v' blocks of 65 per head: v cols 0..63, ones col 64
            v_sb = [cp.tile([128, HC * 65], dt.bfloat16, tag=f"v_sb{i}", name=f"v_sb{i}") for i in range(ST)]
            for st in range(ST):
                nc.vector.memset(v_sb[st][:], 1.0)

            # ---- projection helpers ----
            def load_xhalf(xT, xts, half):
                # half-tiles per D-chunk so projections start after half the
                # input bytes and the exp stream ramps during the DMA window
                xr = xT.rearrange("(t p) s -> p t s", p=128)
                for Dti in range(DT):
                    xtile = xp.tile([128, S // 2], dt.bfloat16, tag="xt",
                                    name=f"xt_{Dti}_{half}")
                    nc.sync.dma_start(
                        xtile[:], xr[:, Dti, half * (S // 2):(half + 1) * (S // 2)])
                    xts[Dti][half] = xtile

            def qk_proj(xts, w_sb, dst, b_sb, qc):
                half, off = qc // 2, (qc % 2) * 512
                for pt in range(2):
                    ps = pb.tile([128, 512], dt.float32, tag="pb", name=f"pp_{qc}_{pt}")
                    for Dti in range(DT):
                        nc.tensor.matmul(
                            ps[:],
                            w_sb[:, Dti, pt * 128:(pt + 1) * 128],
                            xts[Dti][half][:, off:off + 512],
                            start=(Dti == 0),
                            stop=(Dti == DT - 1),
                        )
                    nc.vector.tensor_scalar_add(
                        dst[qc][:, pt, :], ps[:], b_sb[:, pt:pt + 1],
                    )

            def v_proj(xts, st_range):
                for st in st_range:
                    half, off = st // 8, (st % 8) * 128
                    ps = pb.tile([128, DC], dt.float32, tag="pb", name=f"vp_{st}")
                    for Dti in range(DT):
                        nc.tensor.matmul(
                            ps[:],
                            xts[Dti][half][:, off:off + 128],
                            wv_sb[:, Dti, :],
                            start=(Dti == 0),
                            stop=(Dti == DT - 1),
                        )
                    # v_sb block h: cols h*65..h*65+63 = v + bias; col h*65+64 stays 1.0
                    nc.vector.tensor_tensor(
                        v_sb[st].rearrange("p (h c) -> p h c", c=65)[:, :, 0:64],
                        ps.rearrange("p (h d) -> p h d", d=HD),
                        bv_sb.rearrange("p (h d) -> p h d", d=HD),
                        ALU.add,
                    )

            # ---- attention + output projection, software-pipelined over q-chunks:
            # qc's normalize/out-proj tail is traced AFTER qc+1's attention so the
            # next q-chunk's PSUM/ACT stream never waits on the tail chain.
            qc_state = {}

            def scores_exp(qc, kvb, h):
                pt, lo = h // 2, (h % 2) * 64
                scp = pa.tile([128, 1024], dt.float32, tag="pa", name=f"sc_{qc}_{kvb}_{h}")
                for j in range(2):
                    kt = kvb * 2 + j
                    nc.tensor.matmul(
                        scp[:, j * 512:(j + 1) * 512],
                        kT_sb[kt // 4][lo:lo + 64, pt, (kt % 4) * 128:(kt % 4 + 1) * 128],
                        qT_sb[qc][lo:lo + 64, pt, :],
                        start=True,
                        stop=True,
                    )
                ex = ep.tile([128, 1024], dt.bfloat16, tag="ex", name=f"ex_{qc}_{kvb}_{h}")
                nc.scalar.activation(ex[:], scp[:], AF.Exp, scale=0.125)
                return ex

            def attnT_mm(qc, kvb, h, psA, ex):
                for j in range(2):
                    kt = kvb * 2 + j
                    nc.tensor.matmul(
                        psA[h][0:65, :],
                        v_sb[kt][:, h * 65:h * 65 + 65],
                        ex[:, j * 512:(j + 1) * 512],
                        start=(kvb == 0 and j == 0),
                        stop=(kvb == 7 and j == 1),
                    )

            def attention(qc, kvb_range, psA, pre_ex=None):
                for kvb in kvb_range:  # kv blocks of 2 kv-tiles
                    for h in range(HC):
                        key = (kvb, h)
                        if pre_ex and key in pre_ex:
                            ex = pre_ex.pop(key)
                        else:
                            ex = scores_exp(qc, kvb, h)
                        attnT_mm(qc, kvb, h, psA, ex)
                if kvb_range[-1] == 7:
                    # drain PSUM accumulators to SBUF right away (frees the
                    # banks): unnormalized attn rows + sums row
                    rc = rcp.tile([128, HC, 512], dt.float32r, tag="rc")
                    at = atp.tile([64, HC, 512], dt.bfloat16, tag="at")
                    # sums rows first: they gate the reciprocal chain
                    for h in range(HC):
                        nc.vector.tensor_copy(rc[64:65, h, :], psA[h][64:65, :])
                    for h in range(HC):
                        nc.vector.tensor_copy(at[:, h, :], psA[h][0:64, :])
                    qc_state[qc] = (rc, at)

            def tail(qc, last=False):
                rc, at = qc_state.pop(qc)
                # reciprocals: exp(-ln(x)) keeps everything on the exp/ln table set
                nc.scalar.activation(rc[64:65, :, :], rc[64:65, :, :], AF.Ln)
                nc.scalar.activation(rc[64:65, :, :], rc[64:65, :, :], AF.Exp, scale=-1.0)
                for h in range(HC):
                    pbc = pb.tile([128, 512], dt.float32, tag="pb", name=f"bc_{qc}_{h}")
                    nc.tensor.matmul(pbc[:], ones_sb[64:65, :], rc[64:65, h, :],
                                     start=True, stop=True)
                    nc.vector.tensor_tensor(
                        at[:, h, :], at[:, h, :], pbc[0:64, :], ALU.mult,
                    )
                # output projection for this q-chunk's 4 s-tiles
                for sl in range(4):
                    st = qc * 4 + sl
                    o_sb = op_.tile([128, D], dt.float32, tag="osb")
                    for dc2 in range(2):
                        po = pb.tile([128, 512], dt.float32, tag="pb", name=f"po_{st}_{dc2}")
                        for h in range(HC):
                            nc.tensor.matmul(
                                po[:],
                                at[:, h, sl * 128:(sl + 1) * 128],
                                wo_sb[:, h, dc2 * 512:(dc2 + 1) * 512],
                                start=(h == 0),
                                stop=(h == HC - 1),
                            )
                        if last:
                            # end tail: ACT is idle; use it for the copies
                            nc.scalar.copy(o_sb[:, dc2 * 512:(dc2 + 1) * 512], po[:])
                        else:
                            nc.vector.tensor_copy(o_sb[:, dc2 * 512:(dc2 + 1) * 512], po[:])
                        # each half ships as soon as it's staged
                        nc.sync.dma_start(
                            out_t[st][:, dc2 * 512:(dc2 + 1) * 512],
                            o_sb[:, dc2 * 512:(dc2 + 1) * 512])

            # trace order chosen so the exp stream (the ACT bottleneck) starts
            # as early as possible: first halves of xk/xq land first, feeding
            # K(qc0,1)+Q(qc0) and the first half of qc0's scores/exp while the
            # second halves and xv are still streaming; V and the remaining Q
            # chunks fill PE gaps under the ACT-bound attention stream.
            xk_ts = [[None, None] for _ in range(DT)]
            xq_ts = [[None, None] for _ in range(DT)]
            xv_ts = [[None, None] for _ in range(DT)]
            load_xhalf(xkT, xk_ts, 0)
            nc.sync.dma_start(wq_sb[:], wq.rearrange("(t p) m -> p t m", p=128))
            nc.sync.dma_start(bq_sb[:], bq[:])
            load_xhalf(xqT, xq_ts, 0)
            qk_proj(xk_ts, wk_sb, kT_sb, bk_sb, 0)
            qk_proj(xk_ts, wk_sb, kT_sb, bk_sb, 1)
            qk_proj(xq_ts, wq_sb, qT_sb, bq_sb, 0)
            pre_ex = {}
            for kvb in range(4):   # needs only kT_sb[0..1] (xk half 0)
                for h in range(HC):
                    pre_ex[(kvb, h)] = scores_exp(0, kvb, h)
            nc.sync.dma_start(wv_sb[:], wv.rearrange("(t p) m -> p t m", p=128))
            nc.sync.dma_start(bv_sb[:], bv[:])
            nc.sync.dma_start(wo_sb[:], wo[:])
            nc.sync.dma_start(ones_sb[:], ones[:])
            load_xhalf(xvT, xv_ts, 0)
            v_proj(xv_ts, range(0, 8))
            load_xhalf(xkT, xk_ts, 1)
            load_xhalf(xqT, xq_ts, 1)
            qk_proj(xk_ts, wk_sb, kT_sb, bk_sb, 2)
            qk_proj(xk_ts, wk_sb, kT_sb, bk_sb, 3)
            for kvb in range(4, 8):
                for h in range(HC):
                    pre_ex[(kvb, h)] = scores_exp(0, kvb, h)
            for qc in range(1, QC):
                qk_proj(xq_ts, wq_sb, qT_sb, bq_sb, qc)
            load_xhalf(xvT, xv_ts, 1)
            v_proj(xv_ts, range(8, ST))

            def alloc_psA(qc):
                return [pb.tile([128, 512], dt.float32, tag="pb", name=f"att_{qc}_{h}")
                        for h in range(HC)]

            psA = alloc_psA(0)
            attention(0, range(0, 8), psA, pre_ex=pre_ex)
            for qc in range(1, QC):
                psA = alloc_psA(qc)
                attention(qc, range(0, 8), psA)
                tail(qc - 1)
            tail(QC - 1, last=True)

    nc.finalize()
    return nc


def _get_program():
    global _PROGRAM
    if _PROGRAM is None:
        _PROGRAM = _build_program()
    return _PROGRAM


def _prep_core_inputs(x_q, x_k, x_v, wq, bq, wk, bk, wv, bv, wo):
    """Build the 8 per-core input dicts (host-side shard + cast)."""
    ones_np = np.ones((128, 128), np.float32)
    xT = {}
    for b in range(2):
        xT[b] = (
            np.ascontiguousarray(x_q[b].T).astype(BF16),
            np.ascontiguousarray(x_k[b].T).astype(BF16),
            np.ascontiguousarray(x_v[b].T).astype(BF16),
        )
    in_maps = []
    for c in range(NCORES):
        b, g = c // 4, c % 4
        sl = slice(g * DC, (g + 1) * DC)
        wo_c = np.ascontiguousarray(
            wo[sl, :].reshape(HC, HD, D).transpose(1, 0, 2)
        ).astype(BF16)
        in_maps.append({
            "xqT": xT[b][0],
            "xkT": xT[b][1],
            "xvT": xT[b][2],
            "wq": wq[:, sl].astype(BF16),
            "wk": wk[:, sl].astype(BF16),
            "wv": wv[:, sl].astype(BF16),
            "wo": wo_c,
            "bq": np.ascontiguousarray(bq[sl].reshape(2, 128).T).astype(np.float32),
            "bk": np.ascontiguousarray(bk[sl].reshape(2, 128).T).astype(np.float32),
            "bv": np.broadcast_to(bv[sl], (128, DC)).astype(np.float32).copy(),
            "ones": ones_np,
        })
    return in_maps


def kernel(x_q, x_k, x_v, wq, bq, wk, bk, wv, bv, wo, bo):
    from concourse.bass_utils import run_bass_kernel_spmd

    x_q = np.asarray(x_q, np.float32)
    x_k = np.asarray(x_k, np.float32)
    x_v = np.asarray(x_v, np.float32)
    wq = np.asarray(wq, np.float32)
    wk = np.asarray(wk, np.float32)
    wv = np.asarray(wv, np.float32)
    wo = np.asarray(wo, np.float32)
    bq = np.asarray(bq, np.float32)
    bk = np.asarray(bk, np.float32)
    bv = np.asarray(bv, np.float32)
    bo = np.asarray(bo, np.float32)

    nc = _get_program()
    in_maps = _prep_core_inputs(x_q, x_k, x_v, wq, bq, wk, bk, wv, bv, wo)
    res = run_bass_kernel_spmd(nc, in_maps, list(range(NCORES)))

    out = np.zeros((2, S, D), np.float32)
    for c in range(NCORES):
        out[c // 4] += res.results[c]["out"]
    out += bo
    return out



# revision 6
# speedup vs baseline: 1.1582x; 1.1582x over previous
"""Trainium2 Bass kernel for nn_MultiHeadAttention (B=2, S=2048, D=1024, H=16).

Sharding: 8 cores = 2 batches x 4 head-groups. Core c handles batch c//4 and
heads [4*(c%4), 4*(c%4)+4). Each core computes its 4 heads' attention plus the
row-slice of the output projection; the host sums the 4 partial outputs per
batch and adds the output bias.

Per-core structure (all matmuls bf16, fp32 PSUM):
  - qT/kT in [head_dim, seq] layout (host provides x^T), packed 2 heads per
    128-partition tile. wq is host-prescaled by 16*log2(e) so the score PSUM
    is 128*log2(e)*scores/8 directly.
  - scoresT[kv, q] = kT.T @ qT per (head, kv-tile-pair, q-chunk); exp2 on
    ScalarE reading PSUM (scale=ln2/128, bias=-4*ln2 folded into the
    activation; the 2^-4 shift cancels in normalization and keeps bf16
    exponents comfortable).
  - attn[q, d] in q-major layout: attnT'[q, d+1] = ex_slice.T @ [v | 1]
    accumulated over the 16 kv tiles in PSUM (kv innermost so only one
    2KB accumulation region is live per (head, q-tile)); col 64 collects
    the softmax denominators.
  - 1/sums via exact DVE reciprocal; normalize+drain to bf16 on GPSIMD with
    a per-partition scalar multiply.
  - attn -> attnT via DMA XBAR transpose (head pairs stacked to 128 rows);
    out[q, :] = attnT.T @ woPair accumulated over the 2 head pairs (K=128).
"""

import sys

for _p in ("/opt/trn_rl_repo",):
    if _p not in sys.path:
        sys.path.insert(0, _p)

import numpy as np
import ml_dtypes

BF16 = ml_dtypes.bfloat16

S = 2048          # sequence length
D = 1024          # embed dim
HC = 4            # heads per core
HD = 64           # head dim
DC = HC * HD      # per-core projection width (256)
ST = S // 128     # kv tiles (16)
DT = D // 128     # D tiles (8)
QC = S // 512     # q chunks of 512 (4)
NCORES = 8

LOG2E = 1.4426950408889634
PRESCALE = 16.0 * LOG2E   # folded into wq on the host
CSHIFT = 4.0              # ex = exp(s/8) * 2^-CSHIFT

_PROGRAM = None


def _build_program():
    import concourse.mybir as mybir
    import concourse.tile as tile
    from concourse import bacc

    dt = mybir.dt
    AF = mybir.ActivationFunctionType
    ALU = mybir.AluOpType

    nc = bacc.Bacc()

    xqT = nc.declare_dram_parameter("xqT", [D, S], dt.bfloat16, isOutput=False)
    xkT = nc.declare_dram_parameter("xkT", [D, S], dt.bfloat16, isOutput=False)
    xvT = nc.declare_dram_parameter("xvT", [D, S], dt.bfloat16, isOutput=False)
    wq = nc.declare_dram_parameter("wq", [D, DC], dt.bfloat16, isOutput=False)
    wk = nc.declare_dram_parameter("wk", [D, DC], dt.bfloat16, isOutput=False)
    wv = nc.declare_dram_parameter("wv", [D, DC], dt.bfloat16, isOutput=False)
    # wo pairs: [pair, 128 = (2 heads x 64 hd), D]
    woP = nc.declare_dram_parameter("woP", [128, 2, D], dt.bfloat16, isOutput=False)
    bq = nc.declare_dram_parameter("bq", [128, 2], dt.float32, isOutput=False)
    bk = nc.declare_dram_parameter("bk", [128, 2], dt.float32, isOutput=False)
    bv = nc.declare_dram_parameter("bv", [128, DC], dt.float32, isOutput=False)
    out = nc.declare_dram_parameter("out", [S, D], dt.float32, isOutput=True)

    out_t = out.rearrange("(t p) d -> t p d", p=128)

    EXP_SCALE = float(np.log(2.0) / 128.0)
    EXP_BIAS = float(-CSHIFT * np.log(2.0))

    with tile.TileContext(nc) as tc:
        with (
            tc.tile_pool(name="const", bufs=1) as cp,
            tc.tile_pool(name="xt", bufs=26) as xp,
            tc.tile_pool(name="expp", bufs=28) as ep,
            tc.tile_pool(name="atp", bufs=6) as atp,
            tc.tile_pool(name="attp", bufs=6) as atpT,
            tc.tile_pool(name="rcp", bufs=8) as rcp,
            tc.tile_pool(name="outp", bufs=4) as op_,
            tc.tile_pool(name="pa", bufs=2, space="PSUM") as pa,
            tc.tile_pool(name="pv", bufs=2, space="PSUM") as pv,
            tc.tile_pool(name="pb", bufs=2, space="PSUM") as pb,
        ):
            # ---- constants ----
            wq_sb = cp.tile([128, DT, DC], dt.bfloat16, tag="wq_sb")
            wk_sb = cp.tile([128, DT, DC], dt.bfloat16, tag="wk_sb")
            wv_sb = cp.tile([128, DT, DC], dt.bfloat16, tag="wv_sb")
            wo_sb = cp.tile([128, 2, D], dt.bfloat16, tag="wo_sb")
            bq_sb = cp.tile([128, 2], dt.float32, tag="bq_sb")
            bk_sb = cp.tile([128, 2], dt.float32, tag="bk_sb")
            bv_sb = cp.tile([128, HC, HD], dt.float32, tag="bv_sb")
            ebias_sb = cp.tile([128, 1], dt.float32, tag="ebias_sb")
            nc.gpsimd.memset(ebias_sb[:], EXP_BIAS)
            nc.sync.dma_start(wk_sb[:], wk.rearrange("(t p) m -> p t m", p=128))
            nc.sync.dma_start(bk_sb[:], bk[:])

            qT_sb = [cp.tile([128, 2, 512], dt.bfloat16, tag=f"qT_sb{i}", name=f"qT_sb{i}") for i in range(QC)]
            kT_sb = [cp.tile([128, 2, 512], dt.bfloat16, tag=f"kT_sb{i}", name=f"kT_sb{i}") for i in range(QC)]
            # v' per kv tile: [128 kv, head, 64 v + ones col]
            v_sb = [cp.tile([128, HC, 65], dt.bfloat16, tag=f"v_sb{i}", name=f"v_sb{i}") for i in range(ST)]
            for st in range(ST):
                nc.gpsimd.memset(v_sb[st][:, :, 64:65], 1.0)

            # ---- projection helpers ----
            def load_xhalf(xT, xts, half):
                xr = xT.rearrange("(t p) s -> p t s", p=128)
                for Dti in range(DT):
                    xtile = xp.tile([128, S // 2], dt.bfloat16, tag="xt",
                                    name=f"xt_{Dti}_{half}")
                    nc.sync.dma_start(
                        xtile[:], xr[:, Dti, half * (S // 2):(half + 1) * (S // 2)])
                    xts[Dti][half] = xtile

            def qk_proj(xts, w_sb, dst, b_sb, qc):
                half, off = qc // 2, (qc % 2) * 512
                for pt in range(2):
                    ps = pb.tile([128, 512], dt.float32, tag="pb", name=f"pp_{qc}_{pt}")
                    for Dti in range(DT):
                        nc.tensor.matmul(
                            ps[:],
                            w_sb[:, Dti, pt * 128:(pt + 1) * 128],
                            xts[Dti][half][:, off:off + 512],
                            start=(Dti == 0),
                            stop=(Dti == DT - 1),
                        )
                    nc.vector.tensor_scalar_add(
                        dst[qc][:, pt, :], ps[:], b_sb[:, pt:pt + 1],
                    )

            def v_proj(xts, st_range):
                for st in st_range:
                    half, off = st // 8, (st % 8) * 128
                    ps = pv.tile([128, DC], dt.float32, tag="pv", name=f"vp_{st}")
                    for Dti in range(DT):
                        nc.tensor.matmul(
                            ps[:],
                            xts[Dti][half][:, off:off + 128],
                            wv_sb[:, Dti, :],
                            start=(Dti == 0),
                            stop=(Dti == DT - 1),
                        )
                    nc.vector.tensor_tensor(
                        v_sb[st][:, :, 0:64],
                        ps.rearrange("p (h d) -> p h d", d=HD),
                        bv_sb[:],
                        ALU.add,
                    )

            # ---- attention ----
            ex_tiles = {}

            def scores_exp(qc, h, m):
                """Scores+exp for q-chunk qc, head h, kv-tile pair m (kv tiles 2m, 2m+1)."""
                pt, lo = h // 2, (h % 2) * 64
                scp = pa.tile([128, 2, 512], dt.float32, tag="pa", name=f"sc_{qc}_{h}_{m}")
                for j in range(2):
                    kt = m * 2 + j
                    nc.tensor.matmul(
                        scp[:, j, :],
                        kT_sb[kt // 4][lo:lo + 64, pt, (kt % 4) * 128:(kt % 4 + 1) * 128],
                        qT_sb[qc][lo:lo + 64, pt, :],
                        start=True,
                        stop=True,
                    )
                ex = ep.tile([128, 2, 512], dt.bfloat16, tag="ex", name=f"ex_{qc}_{h}_{m}")
                nc.scalar.activation(ex[:], scp[:], AF.Exp, scale=EXP_SCALE,
                                     bias=ebias_sb[:])
                ex_tiles[(qc, h, m)] = ex

            def attn_head(qc, h, at_pair):
                """attnV + normalize for (qc, h); writes at_pair[qs][:, (h%2)*64:...]."""
                exs = [ex_tiles.pop((qc, h, m)) for m in range(8)]
                rc = rcp.tile([128, 4], dt.float32, tag="rc", name=f"rc_{qc}_{h}")
                for qs in range(4):
                    pA = pv.tile([128, 512], dt.float32, tag="pv", name=f"att_{qc}_{h}_{qs}")
                    for kt in range(ST):
                        nc.tensor.matmul(
                            pA[:, 0:65],
                            exs[kt // 2][:, kt % 2, qs * 128:(qs + 1) * 128],
                            v_sb[kt][:, h, :],
                            start=(kt == 0),
                            stop=(kt == ST - 1),
                        )
                    nc.vector.reciprocal(rc[:, qs:qs + 1], pA[:, 64:65])
                    nc.vector.tensor_scalar_mul(
                        at_pair[qs][:, (h % 2) * 64:(h % 2) * 64 + 64],
                        pA[:, 0:64],
                        rc[:, qs:qs + 1],
                    )

            def attention(qc):
                atT = {}
                for pair in range(2):
                    at_pair = [
                        atp.tile([128, 128], dt.bfloat16, tag="at",
                                 name=f"at_{qc}_{pair}_{qs}")
                        for qs in range(4)
                    ]
                    for hh in range(2):
                        h = pair * 2 + hh
                        for m in range(8):
                            if (qc, h, m) not in ex_tiles:
                                scores_exp(qc, h, m)
                        attn_head(qc, h, at_pair)
                    for qs in range(4):
                        t = atpT.tile([128, 128], dt.bfloat16, tag="atT",
                                      name=f"atT_{qc}_{pair}_{qs}")
                        nc.sync.dma_start(t[:], at_pair[qs][:], transpose=True)
                        atT[(pair, qs)] = t
                # output projection per q-tile
                for qs in range(4):
                    st = qc * 4 + qs
                    o_sb = op_.tile([128, D], dt.float32, tag="osb")
                    for dc2 in range(2):
                        po = pb.tile([128, 512], dt.float32, tag="pb", name=f"po_{st}_{dc2}")
                        for pair in range(2):
                            nc.tensor.matmul(
                                po[:],
                                atT[(pair, qs)][:],
                                wo_sb[:, pair, dc2 * 512:(dc2 + 1) * 512],
                                start=(pair == 0),
                                stop=(pair == 1),
                            )
                        nc.vector.tensor_copy(o_sb[:, dc2 * 512:(dc2 + 1) * 512], po[:])
                        nc.sync.dma_start(
                            out_t[st][:, dc2 * 512:(dc2 + 1) * 512],
                            o_sb[:, dc2 * 512:(dc2 + 1) * 512])

            # ---- trace order ----
            xk_ts = [[None, None] for _ in range(DT)]
            xq_ts = [[None, None] for _ in range(DT)]
            xv_ts = [[None, None] for _ in range(DT)]
            load_xhalf(xkT, xk_ts, 0)
            nc.sync.dma_start(wq_sb[:], wq.rearrange("(t p) m -> p t m", p=128))
            nc.sync.dma_start(bq_sb[:], bq[:])
            load_xhalf(xqT, xq_ts, 0)
            qk_proj(xk_ts, wk_sb, kT_sb, bk_sb, 0)
            qk_proj(xk_ts, wk_sb, kT_sb, bk_sb, 1)
            qk_proj(xq_ts, wq_sb, qT_sb, bq_sb, 0)
            # pre-exp on kv 0..1023 for qc0 h0/h1 while the rest streams in
            for h in range(2):
                for m in range(4):
                    scores_exp(0, h, m)
            nc.sync.dma_start(wv_sb[:], wv.rearrange("(t p) m -> p t m", p=128))
            nc.sync.dma_start(bv_sb[:], bv.rearrange("p (h d) -> p h d", d=HD))
            nc.sync.dma_start(wo_sb[:], woP[:])
            load_xhalf(xvT, xv_ts, 0)
            v_proj(xv_ts, range(0, 8))
            load_xhalf(xkT, xk_ts, 1)
            load_xhalf(xqT, xq_ts, 1)
            qk_proj(xk_ts, wk_sb, kT_sb, bk_sb, 2)
            qk_proj(xk_ts, wk_sb, kT_sb, bk_sb, 3)
            for h in range(2, 4):
                for m in range(4):
                    scores_exp(0, h, m)
            for qc in range(1, QC):
                qk_proj(xq_ts, wq_sb, qT_sb, bq_sb, qc)
            load_xhalf(xvT, xv_ts, 1)
            v_proj(xv_ts, range(8, ST))

            for qc in range(QC):
                attention(qc)

    nc.finalize()
    return nc


def _get_program():
    global _PROGRAM
    if _PROGRAM is None:
        _PROGRAM = _build_program()
    return _PROGRAM


def _prep_core_inputs(x_q, x_k, x_v, wq, bq, wk, bk, wv, bv, wo):
    """Build the 8 per-core input dicts (host-side shard + cast)."""
    xT = {}
    for b in range(2):
        xT[b] = (
            np.ascontiguousarray(x_q[b].T).astype(BF16),
            np.ascontiguousarray(x_k[b].T).astype(BF16),
            np.ascontiguousarray(x_v[b].T).astype(BF16),
        )
    wq_s = (wq * PRESCALE).astype(BF16)
    bq_s = (bq * PRESCALE).astype(np.float32)
    in_maps = []
    for c in range(NCORES):
        b, g = c // 4, c % 4
        sl = slice(g * DC, (g + 1) * DC)
        # wo rows for this head group, head-pairs stacked on partitions:
        # woP[p, pair, :] with p = (h_in_pair * 64 + hd)
        wo_c = np.ascontiguousarray(
            wo[sl, :].reshape(2, 2 * HD, D).transpose(1, 0, 2)
        ).astype(BF16)
        in_maps.append({
            "xqT": xT[b][0],
            "xkT": xT[b][1],
            "xvT": xT[b][2],
            "wq": wq_s[:, sl],
            "wk": wk[:, sl].astype(BF16),
            "wv": wv[:, sl].astype(BF16),
            "woP": wo_c,
            "bq": np.ascontiguousarray(bq_s[sl].reshape(2, 128).T).astype(np.float32),
            "bk": np.ascontiguousarray(bk[sl].reshape(2, 128).T).astype(np.float32),
            "bv": np.broadcast_to(bv[sl], (128, DC)).astype(np.float32).copy(),
        })
    return in_maps


def kernel(x_q, x_k, x_v, wq, bq, wk, bk, wv, bv, wo, bo):
    from concourse.bass_utils import run_bass_kernel_spmd

    x_q = np.asarray(x_q, np.float32)
    x_k = np.asarray(x_k, np.float32)
    x_v = np.asarray(x_v, np.float32)
    wq = np.asarray(wq, np.float32)
    wk = np.asarray(wk, np.float32)
    wv = np.asarray(wv, np.float32)
    wo = np.asarray(wo, np.float32)
    bq = np.asarray(bq, np.float32)
    bk = np.asarray(bk, np.float32)
    bv = np.asarray(bv, np.float32)
    bo = np.asarray(bo, np.float32)

    nc = _get_program()
    in_maps = _prep_core_inputs(x_q, x_k, x_v, wq, bq, wk, bk, wv, bv, wo)
    res = run_bass_kernel_spmd(nc, in_maps, list(range(NCORES)))

    out = np.zeros((2, S, D), np.float32)
    for c in range(NCORES):
        out[c // 4] += res.results[c]["out"]
    out += bo
    return out
